# revision 2
# baseline (speedup 1.0000x reference)
"""Trainium2 Bass kernel for nn_ComplexMixture: weighted complex Gram matrices.

Reference (per batch b, inputs real/imag [B,T,D] f32, weight [B,T,1] f32):
    out_r[b] = sum_t w[b,t] * (r_t r_t^T + i_t i_t^T)   (symmetric)
    out_i[b] = sum_t w[b,t] * (i_t r_t^T - r_t i_t^T)   (antisymmetric)
with B=64, T=256, D=512; outputs (out_r, out_i), each [B, D, D] f32.

Pure data-parallel over 8 NeuronCores (8 batches per core). Per batch:
  - T=256 lives on partitions as KT=2 K-tiles; a = fp16(sqrt(w)*r),
    c = fp16(sqrt(w)*i), nct = -c, all built on the ACT (scalar) engine
    with a per-partition scale AP. ACT scaling measured ~3x faster
    end-to-end than DVE/gpsimd scaling: it decouples the scale stage from
    the DVE eviction stage so consecutive loop iterations overlap.
  - Only the upper block-trapezoid is computed (out_r symmetric, out_i
    antisymmetric; host mirrors the lower blocks): row-block mi covers
    cols [mi*128, 512), widths 512/384/256/128.
  - Per row-block: pr += a_m^T a_n + c_m^T c_n (4 fp16 matmuls, exact
    fp32 PSUM accum), pi += a_m^T (-c)_n + c_m^T a_n (4 matmuls).
  - DVE evicts PSUM->SBUF staging with fp16 downconvert (global rel err
    ~3.6e-4, far under the 2e-2 gate; fp16 halves output DMA bytes).
  - One SWDGE (gpsimd) DMA per batch stores the packed [128, 2560] fp16
    staging tile contiguously (5KB/partition). HWDGE (sync) loads inputs.
Host: unpack fp16 -> f32, mirror lower blocks (r: +transpose, i: -transpose).

Measured (reps-differencing, tiny-I/O controlled A/B): this config ~46 us
vs ~178 us for the f32-output DVE-scaled baseline structure.
"""
import numpy as np
from contextlib import ExitStack

import concourse.bacc as bacc
import concourse.tile as tile
from concourse import mybir
from concourse.bass_utils import run_bass_kernel_spmd

F32 = mybir.dt.float32
FP16 = mybir.dt.float16

N_CORES = 8
B_FULL = 64
BPC = B_FULL // N_CORES  # batches per core
T, D = 256, 512
KT = T // 128            # K tiles per batch
MT = D // 128            # output row blocks

WIDTHS = [D - 128 * mi for mi in range(MT)]   # 512, 384, 256, 128
OFFS = [sum(WIDTHS[:j]) for j in range(MT)]   # 0, 512, 896, 1152
PK = sum(WIDTHS)                              # 1280 packed cols per matrix


def build_nc(reps: int = 1, unroll: int = 1,
             inp_bufs=4, wgt_bufs=3, outp_bufs=4, ps_bufs=8):
    """Build + compile the per-core program. reps>1 wraps the body in a
    hardware loop (timing only; output is idempotent). unroll>1 python-
    unrolls instead (for the timeline simulator, which can't run For_i)."""
    nc = bacc.Bacc("TRN2", target_bir_lowering=False, debug=False)
    real = nc.dram_tensor("real", [BPC, T, D], F32, kind="ExternalInput").ap()
    imag = nc.dram_tensor("imag", [BPC, T, D], F32, kind="ExternalInput").ap()
    weight = nc.dram_tensor("weight", [BPC, T, 1], F32, kind="ExternalInput").ap()
    out_p = nc.dram_tensor("out_p", [BPC, 128, 2 * PK], FP16,
                           kind="ExternalOutput").ap()

    with tile.TileContext(nc) as tc, ExitStack() as ctx:
        wp = ctx.enter_context(tc.tile_pool(name="wp", bufs=1))
        inp = ctx.enter_context(tc.tile_pool(name="inp", bufs=inp_bufs))
        wgt = ctx.enter_context(tc.tile_pool(name="wgt", bufs=wgt_bufs))
        outp = ctx.enter_context(tc.tile_pool(name="outp", bufs=outp_bufs))
        psp = ctx.enter_context(tc.tile_pool(name="psp", bufs=ps_bufs, space="PSUM"))

        def body(_iv=None):
            # W[p, b*KT+kt] = w[b, kt*128+p]; SW = sqrt(W), NSW = -sqrt(W)
            W = wp.tile([128, BPC * KT], F32, tag="W")
            nc.sync.dma_start(
                W[:], weight.rearrange("b (kt p) o -> p (b kt o)", kt=KT, p=128)
            )
            SW = wp.tile([128, BPC * KT], F32, tag="SW")
            nc.scalar.activation(SW[:], W[:], mybir.ActivationFunctionType.Sqrt)
            NSW = wp.tile([128, BPC * KT], F32, tag="NSW")
            nc.vector.tensor_scalar_mul(NSW[:], SW[:], -1.0)

            for b in range(BPC):
                rt = inp.tile([128, KT * D], F32, tag="rt")
                it = inp.tile([128, KT * D], F32, tag="it")
                nc.sync.dma_start(
                    rt[:].rearrange("p (kt d) -> p kt d", kt=KT),
                    real[b].rearrange("(kt p) d -> p kt d", kt=KT, p=128),
                )
                nc.sync.dma_start(
                    it[:].rearrange("p (kt d) -> p kt d", kt=KT),
                    imag[b].rearrange("(kt p) d -> p kt d", kt=KT, p=128),
                )

                a = wgt.tile([128, KT * D], FP16, tag="a")    # sqrt(w)*r
                c = wgt.tile([128, KT * D], FP16, tag="c")    # sqrt(w)*i
                nct = wgt.tile([128, KT * D], FP16, tag="nc")  # -sqrt(w)*i
                for kt in range(KT):
                    sl = slice(kt * D, (kt + 1) * D)
                    ws = SW[:, b * KT + kt:b * KT + kt + 1]
                    nws = NSW[:, b * KT + kt:b * KT + kt + 1]
                    nc.scalar.mul(a[:, sl], rt[:, sl], ws)
                    nc.scalar.mul(c[:, sl], it[:, sl], ws)
                    nc.scalar.mul(nct[:, sl], it[:, sl], nws)

                stt = outp.tile([128, 2 * PK], FP16, tag="st")
                for mi in range(MT):
                    w_mi = WIDTHS[mi]
                    col0 = mi * 128
                    pr = psp.tile([128, w_mi], F32, tag="ps",
                                  padded_shape=[128, D], name="pr")
                    pi = psp.tile([128, w_mi], F32, tag="ps",
                                  padded_shape=[128, D], name="pi")
                    for kt in range(KT):
                        m = slice(kt * D + col0, kt * D + col0 + 128)
                        n = slice(kt * D + col0, kt * D + col0 + w_mi)
                        st_ = kt == 0
                        sp = kt == KT - 1
                        nc.tensor.matmul(pr[:], a[:, m], a[:, n], start=st_, stop=False)
                        nc.tensor.matmul(pi[:], a[:, m], nct[:, n], start=st_, stop=False)
                        nc.tensor.matmul(pi[:], c[:, m], a[:, n], start=False, stop=sp)
                        nc.tensor.matmul(pr[:], c[:, m], c[:, n], start=False, stop=sp)
                    nc.vector.tensor_copy(stt[:, OFFS[mi]:OFFS[mi] + w_mi], pr[:])
                    nc.vector.tensor_copy(
                        stt[:, PK + OFFS[mi]:PK + OFFS[mi] + w_mi], pi[:])
                nc.gpsimd.dma_start(out_p[b], stt[:])

        if unroll > 1:
            for _ in range(unroll):
                body()
        elif reps == 1:
            body()
        else:
            with tc.For_i(0, reps, 1) as iv:
                body(iv)

    nc.compile()
    return nc


_NC_CACHE = {}


def _get_nc(reps: int = 1):
    if reps not in _NC_CACHE:
        _NC_CACHE[reps] = build_nc(reps=reps)
    return _NC_CACHE[reps]


def _unpack(res_list):
    """Per-core out_p [BPC, 128, 2*PK] fp16 -> full f32 (out_r, out_i)."""
    p = np.concatenate(res_list, axis=0)  # [B, 128, 2*PK]
    out_r = np.empty((B_FULL, D, D), np.float32)
    out_i = np.empty((B_FULL, D, D), np.float32)
    for mi in range(MT):
        w = WIDTHS[mi]
        rs = slice(mi * 128, (mi + 1) * 128)
        cs = slice(mi * 128, mi * 128 + w)
        out_r[:, rs, cs] = p[:, :, OFFS[mi]:OFFS[mi] + w]
        out_i[:, rs, cs] = p[:, :, PK + OFFS[mi]:PK + OFFS[mi] + w]
    for mi in range(1, MT):
        for nj in range(mi):
            rs = slice(mi * 128, (mi + 1) * 128)
            cs = slice(nj * 128, (nj + 1) * 128)
            out_r[:, rs, cs] = out_r[:, cs, rs].transpose(0, 2, 1)
            out_i[:, rs, cs] = -out_i[:, cs, rs].transpose(0, 2, 1)
    return out_r, out_i


def kernel(real, imag, weight):
    real = np.ascontiguousarray(np.asarray(real, dtype=np.float32))
    imag = np.ascontiguousarray(np.asarray(imag, dtype=np.float32))
    weight = np.ascontiguousarray(np.asarray(weight, dtype=np.float32))
    assert real.shape == (B_FULL, T, D) and weight.shape == (B_FULL, T, 1)

    nc = _get_nc()
    in_maps = [
        {
            "real": real[i * BPC:(i + 1) * BPC],
            "imag": imag[i * BPC:(i + 1) * BPC],
            "weight": weight[i * BPC:(i + 1) * BPC],
        }
        for i in range(N_CORES)
    ]
    res = run_bass_kernel_spmd(nc, in_maps, list(range(N_CORES)))
    return _unpack([res.results[i]["out_p"] for i in range(N_CORES)])
